# revision 18
# baseline (speedup 1.0000x reference)
"""GumbelTopK Trainium2 kernel.

Computes, row-wise along the last dim (M=2048):
    gumbel    = -log(-log(U + EPS) + EPS)
    x         = logits + gumbel                  (TAU = 1)
    probs     = softmax(x)
    thr       = 30th largest of probs
    out       = probs * sigmoid((probs - thr) / SOFTNESS)

Sharding: fully data-parallel. C=64 leading dim split across 8 cores
(8 x 512 = 4096 rows of 2048 per core, processed as 32 tiles of 128
partitions x 2048).

Per-tile engine split:
  ScalarE (ACT): l = ln(U+eps); s = ln(-l+eps); e = exp(x) [+ fused row
                 sum Z]; mask = sigmoid(e*(1/(softness*Z)) - thr_e/(softness*Z))
  GPSIMD (POOL): x = logits - s; out = (e * (1/Z)) * mask
  VectorE (DVE): exact top-30 threshold in e-space: 4 rounds of max8
                 with match_replace(0.0) between rounds (e > 0 always, so
                 zeroing removes values from subsequent rounds). The 30th
                 largest = element 5 of round 4 (ranks 25..32).
    Softmax needs no max-subtraction: x <= ~23 so exp stays in fp32 range,
    and working in e-space makes the top-k threshold directly usable.
"""

import numpy as np

import concourse.bacc as bacc
import concourse.bass as bass
import concourse.mybir as mybir
import concourse.tile as tile
from concourse.bass_utils import run_bass_kernel_spmd

C, L, M = 64, 512, 2048
N_CORES = 8
K = 30
EPS = 1e-20
SOFTNESS = 0.01

ROWS_PER_CORE = (C // N_CORES) * L  # 4096
P = 128
NTILES = ROWS_PER_CORE // P  # 32

F32 = mybir.dt.float32
AF = mybir.ActivationFunctionType
OP = mybir.AluOpType

_cache = {}


def _build(n_tiles=NTILES):
    rows_total = n_tiles * P
    # Bacc (not raw Bass): its generate_event_semaphores pass splits multi-wait
    # instructions, which activation-with-AP-bias (S3D3_AC struct) requires.
    nc = bacc.Bacc("TRN2", debug=False)
    logits_d = nc.dram_tensor("logits", [rows_total, M], F32, kind="ExternalInput")
    u_d = nc.dram_tensor("u", [rows_total, M], F32, kind="ExternalInput")
    out_d = nc.dram_tensor("out", [rows_total, M], F32, kind="ExternalOutput")

    with tile.TileContext(nc) as tc:
        with (
            tc.tile_pool(name="io", bufs=4) as io,
            tc.tile_pool(name="work", bufs=4) as work,
            tc.tile_pool(name="small", bufs=4) as small,
            tc.tile_pool(name="consts", bufs=1) as consts,
        ):
            eps_t = consts.tile([P, 1], F32)
            nc.vector.memset(eps_t, EPS)
            for i in range(n_tiles):
                rows = slice(i * P, (i + 1) * P)

                u_t = io.tile([P, M], F32, tag="u")
                nc.sync.dma_start(out=u_t, in_=u_d[rows, :])
                lg_t = io.tile([P, M], F32, tag="lg")
                nc.sync.dma_start(out=lg_t, in_=logits_d[rows, :])

                # u := s = ln(-ln(U+eps)+eps) in place; gumbel = -s
                nc.scalar.activation(u_t, u_t, AF.Ln, bias=eps_t, scale=1.0)
                nc.scalar.activation(u_t, u_t, AF.Ln, bias=eps_t, scale=-1.0)

                # lg := x = logits - s in place (on POOL to keep DVE free)
                nc.gpsimd.tensor_sub(lg_t, lg_t, u_t)

                # Z = sum(exp(x)) via fused accumulate; the full-width exp
                # output is a throwaway (written into the mask buffer, which
                # the sigmoid later overwrites).
                mask_t = work.tile([P, M], F32, tag="mask")
                z_t = small.tile([P, 1], F32, tag="z")
                nc.scalar.activation(mask_t, lg_t, AF.Exp, accum_out=z_t)

                # nl = -ln(Z) per row (two tiny ACT ops)
                lnz_t = small.tile([P, 1], F32, tag="lnz")
                nc.scalar.activation(lnz_t, z_t, AF.Ln, bias=eps_t, scale=1.0)
                nl_t = small.tile([P, 1], F32, tag="nl")
                nc.scalar.activation(nl_t, lnz_t, AF.Copy, scale=-1.0)

                # p = exp(x - ln Z) = softmax(x), directly normalized
                p_t = work.tile([P, M], F32, tag="p")
                nc.scalar.activation(p_t, lg_t, AF.Exp, bias=nl_t, scale=1.0)

                # Exact top-30 threshold on p via 4 rounds of max8 +
                # match_replace(0.0) (p > 0, so zeroed values drop out of
                # subsequent rounds).
                m1 = small.tile([P, 8], F32, tag="m1")
                m2 = small.tile([P, 8], F32, tag="m2")
                m3 = small.tile([P, 8], F32, tag="m3")
                m4 = small.tile([P, 8], F32, tag="m4")
                f_t = work.tile([P, M], F32, tag="f")
                nc.vector.max(out=m1, in_=p_t)
                nc.vector.match_replace(
                    out=f_t, in_to_replace=m1, in_values=p_t, imm_value=0.0
                )
                nc.vector.max(out=m2, in_=f_t)
                nc.vector.match_replace(
                    out=f_t, in_to_replace=m2, in_values=f_t, imm_value=0.0
                )
                nc.vector.max(out=m3, in_=f_t)
                nc.vector.match_replace(
                    out=f_t, in_to_replace=m3, in_values=f_t, imm_value=0.0
                )
                nc.vector.max(out=m4, in_=f_t)
                # b = -thr/softness, thr = rank 30 = index 5 of ranks 25..32
                b_t = small.tile([P, 1], F32, tag="b")
                nc.vector.tensor_scalar(
                    b_t,
                    m4[:, K - 24 - 1 : K - 24],
                    -1.0 / SOFTNESS,
                    scalar2=None,
                    op0=OP.mult,
                )

                # mask = sigmoid(p/softness + b)
                nc.scalar.activation(
                    mask_t, p_t, AF.Sigmoid, bias=b_t, scale=1.0 / SOFTNESS
                )

                # out = p * mask (on POOL)
                o_t = io.tile([P, M], F32, tag="o")
                nc.gpsimd.tensor_mul(o_t, p_t, mask_t)
                nc.sync.dma_start(out=out_d[rows, :], in_=o_t)
    nc.compile()
    return nc


def _get_nc():
    if "nc" not in _cache:
        _cache["nc"] = _build()
    return _cache["nc"]


def kernel(logits: np.ndarray, U: np.ndarray) -> np.ndarray:
    assert logits.shape == (C, L, M) and U.shape == (C, L, M)
    lg = np.ascontiguousarray(logits, dtype=np.float32).reshape(
        N_CORES, ROWS_PER_CORE, M
    )
    uu = np.ascontiguousarray(U, dtype=np.float32).reshape(N_CORES, ROWS_PER_CORE, M)
    in_maps = [{"logits": lg[c], "u": uu[c]} for c in range(N_CORES)]
    res = run_bass_kernel_spmd(_get_nc(), in_maps, core_ids=list(range(N_CORES)))
    out = np.stack([r["out"] for r in res.results])
    return out.reshape(C, L, M)
